# revision 5
# baseline (speedup 1.0000x reference)
"""DenseGeneralAqt inference kernel for Trainium2 (8 NeuronCores).

out = (x @ dequant_int8(qkernel)) * qscale,  x:(2,2048,1024) f32,
qkernel:(1024,4096) int8, qscale:(1,4096) f32 -> out:(2,2048,4096) f32.

Strategy: 2D sharding — 4-way over the flattened token axis (M) x 2-way
over features (N). That minimizes per-core input traffic (2.1 MB x +
2.1 MB w, the HBM-bound startup phase). Input marshalling transposes x
to [D, M] (contraction on SBUF partitions) and casts it to fp16, the
same host pass that shards it.

On device: xT rides the Sync HWDGE ring (k-tile 0 first so the first
stationary load is gated only by 256 KB), the int8 weights ride the
Scalar HWDGE ring (k-tile 0 in halves first — the gpsimd SWDGE path
costs ~4 us to first-land, HWDGE ~2 us). Weight k-tiles dequantize to
fp16 on the vector engine in k order, pipelined ahead of PE
consumption. A short PE warm-up (dummy matmuls on zeros) bridges the
DMA-landing window so the HAM clock-gate releases (1.2 -> 2.4 GHz)
close to when real matmuls start. The per-channel scale replicates via
a deferred DRE-broadcast DMA on the (otherwise idle) GpSimd ring and
is fused into the PSUM->SBUF drain on the vector engine.

m-tile pairs x 4 n-tiles sweep all 8 PSUM banks k-outer; the last
sweep runs n-outer so drains overlap the remaining matmuls, its stores
alternate between the Scalar and Sync rings, and the final two banks
drain in column chunks so the last HBM write (and its completion
receipt) is small.
"""

import numpy as np

P = 128
B, S, D, F = 2, 2048, 1024, 4096
N_CORES = 8
MSH, NSH = 4, 2                   # shard grid: 4 m-blocks x 2 n-blocks
M_FULL = B * S                    # 4096 rows
M_CORE = M_FULL // MSH            # 1024 rows per core
N_CORE = F // NSH                 # 2048 cols per core
NT = 512                          # n-tile (one PSUM bank of f32)
WM, WK, WN = M_CORE // P, D // P, N_CORE // NT

_CACHE: dict = {}


def _build():
    import concourse.tile as tile
    from concourse import bacc, mybir

    nc = bacc.Bacc("TRN2", target_bir_lowering=False, debug=False)

    xt_dram = nc.dram_tensor("xt", [D, M_CORE], mybir.dt.float16, kind="ExternalInput")
    w_dram = nc.dram_tensor("w", [D, N_CORE], mybir.dt.int8, kind="ExternalInput")
    s_dram = nc.dram_tensor("s", [1, N_CORE], mybir.dt.float32, kind="ExternalInput")
    o_dram = nc.dram_tensor("o", [M_CORE, N_CORE], mybir.dt.float32, kind="ExternalOutput")

    xt_view = xt_dram[:, :].rearrange("(kt kp) m -> kp kt m", kp=P)  # [128, 8, 1024]

    with tile.TileContext(nc) as tc:
        with (
            tc.tile_pool(name="wi", bufs=1) as wip,
            tc.tile_pool(name="w", bufs=1) as wp,
            tc.tile_pool(name="qs", bufs=1) as qp,
            tc.tile_pool(name="xh", bufs=1) as xhp,
            tc.tile_pool(name="o", bufs=12) as op,
            tc.tile_pool(name="ps", bufs=8, space="PSUM") as pp,
        ):
            # Weights + activations interleaved across BOTH HWDGE rings so
            # HBM delivers them in consumption (k) order. k-tile 0's weights
            # lead in halves — the first 128 KB alone gates the first
            # dequant; everything before the first matmul is latency-bound.
            w_i8 = [
                wip.tile([P, N_CORE], mybir.dt.int8, name=f"wi{kt}", tag=f"wi{kt}")
                for kt in range(WK)
            ]
            wh = N_CORE // 2
            xh = xhp.tile([P, WK, M_CORE], mybir.dt.float16, name="xh", tag="xh")

            def wdma(eng, kt):
                eng.dma_start(w_i8[kt][:], w_dram[kt * P:(kt + 1) * P, :])

            def xdma(eng, a, b):
                eng.dma_start(xh[:, a:b, :], xt_view[:, a:b, :])

            nc.scalar.dma_start(w_i8[0][:, 0:wh], w_dram[0:P, 0:wh])
            xdma(nc.sync, 0, 1)
            nc.scalar.dma_start(w_i8[0][:, wh:N_CORE], w_dram[0:P, wh:N_CORE])
            wdma(nc.sync, 1)
            wdma(nc.scalar, 2)
            xdma(nc.sync, 1, 2)
            wdma(nc.scalar, 3)
            wdma(nc.sync, 4)
            xdma(nc.scalar, 2, 4)
            wdma(nc.sync, 5)
            wdma(nc.scalar, 6)
            xdma(nc.sync, 4, 6)
            wdma(nc.scalar, 7)
            xdma(nc.sync, 6, 8)

            # PE warm-up on zeros while the first loads are in flight: long
            # enough to bridge to the first real matmul (~11 us) so the HAM
            # clock-gate (3.4 us of sustained PE busy) releases first.
            warm = wp.tile([P, NT], mybir.dt.float16, name="warm", tag="warm")
            nc.vector.memset(warm[:], 0)
            warm_ps = pp.tile([P, NT], mybir.dt.float32, name="warm_ps", tag="ps")
            for _ in range(32):
                nc.tensor.matmul(warm_ps[:, 0:P], warm[:, 0:P], warm[:, 0:P])

            # Dequant int8 -> fp16 on the vector engine, in k order; k-tile 0
            # in quarters and k-tile 1 in halves so the earliest matmuls'
            # columns are ready soonest after their bytes land.
            w_sb = [
                wp.tile([P, N_CORE], mybir.dt.float16, name=f"w{kt}", tag=f"w{kt}")
                for kt in range(WK)
            ]
            wq = N_CORE // 4
            for q in range(4):
                nc.vector.tensor_copy(
                    w_sb[0][:, q * wq:(q + 1) * wq], w_i8[0][:, q * wq:(q + 1) * wq]
                )
            nc.vector.tensor_copy(w_sb[1][:, 0:wh], w_i8[1][:, 0:wh])
            nc.vector.tensor_copy(w_sb[1][:, wh:N_CORE], w_i8[1][:, wh:N_CORE])
            cv = [nc.vector.tensor_copy(w_sb[kt][:], w_i8[kt][:]) for kt in range(2, WK)]

            # Scale broadcast (1 MB DRE replication) on the GpSimd SWDGE
            # ring; deferred so its bytes don't starve the critical early
            # loads. Needed only at the first drain (~23 us in).
            qs = qp.tile([P, N_CORE], mybir.dt.float32)
            qs_dma = nc.gpsimd.dma_start(qs[:], s_dram[0:1, :].to_broadcast((P, N_CORE)))
            tile.add_dep_helper(qs_dma.ins, cv[0].ins, reason="defer qs broadcast")

            def drain(mi, nt, ps_tile, eng, chunks=1):
                # PSUM -> (xqscale) -> SBUF -> DRAM. chunks>1 splits the
                # drain column-wise so the final HBM write receipt is small.
                cw = NT // chunks
                for c in range(chunks):
                    ot = op.tile([P, cw], mybir.dt.float32, name=f"o{mi}_{nt}_{c}", tag="o")
                    sl = slice(nt * NT + c * cw, nt * NT + (c + 1) * cw)
                    nc.vector.tensor_mul(ot[:], ps_tile[:, c * cw:(c + 1) * cw], qs[:, sl])
                    e = eng if chunks == 1 else (nc.sync if c % 2 == 0 else nc.scalar)
                    e.dma_start(o_dram[mi * P:(mi + 1) * P, sl], ot[:])

            def mm(ps_tile, kt, mi, nt, first, last):
                nc.tensor.matmul(
                    ps_tile[:],
                    xh[:, kt, mi * P:(mi + 1) * P],
                    w_sb[kt][:, nt * NT:(nt + 1) * NT],
                    start=first,
                    stop=last,
                )

            # m-tile pairs x 4 n-tiles = 8 PSUM banks per k-outer sweep.
            pairs = [(2 * i, 2 * i + 1) for i in range(WM // 2)]
            for pi, pair in enumerate(pairs):
                combos = [(mi, nt) for mi in pair for nt in range(WN)]
                if pi < len(pairs) - 1:
                    # k-outer: consume each weight k-tile across all 8 banks
                    # as soon as it is dequantized.
                    ps = {
                        c: pp.tile([P, NT], mybir.dt.float32, name=f"ps{pi}_{c[0]}_{c[1]}", tag="ps")
                        for c in combos
                    }
                    for kt in range(WK):
                        if pi == 0 and kt <= 1:
                            # n-minor: each n-block's matmuls gate only on
                            # the piecewise dequant of its own columns.
                            order = [(m, n) for n in range(WN) for m in pair]
                        else:
                            order = combos
                        for c in order:
                            mm(ps[c], kt, c[0], c[1], kt == 0, kt == WK - 1)
                    for c in combos:
                        drain(c[0], c[1], ps[c], nc.scalar)
                else:
                    # Last sweep: n-outer so each bank's reduction finishes
                    # early and its drain overlaps the remaining matmuls;
                    # stores alternate rings, and the last two banks drain
                    # in column chunks to shrink the completion tail.
                    for ci, c in enumerate(combos):
                        ps_t = pp.tile([P, NT], mybir.dt.float32, name=f"ps{pi}_{c[0]}_{c[1]}", tag="ps")
                        for kt in range(WK):
                            mm(ps_t, kt, c[0], c[1], kt == 0, kt == WK - 1)
                        chunks = 1 if ci < 6 else (2 if ci == 6 else 4)
                        eng = nc.scalar if ci % 2 == 0 else nc.sync
                        drain(c[0], c[1], ps_t, eng, chunks=chunks)

    nc.compile()
    return nc


def _get_nc():
    if "nc" not in _CACHE:
        _CACHE["nc"] = _build()
    return _CACHE["nc"]


def _run(x, qkernel, qscale, trace=False):
    from concourse.bass_utils import run_bass_kernel_spmd

    x = np.asarray(x, dtype=np.float32).reshape(M_FULL, D)
    xt = np.ascontiguousarray(x.T).astype(np.float16)    # [D, M_FULL]
    w = np.asarray(qkernel)
    if w.dtype != np.int8:
        w = w.astype(np.int8)
    s = np.asarray(qscale, dtype=np.float32).reshape(1, F)

    in_maps = []
    for c in range(N_CORES):
        mb, nb = c % MSH, c // MSH
        in_maps.append({
            "xt": np.ascontiguousarray(xt[:, mb * M_CORE:(mb + 1) * M_CORE]),
            "w": np.ascontiguousarray(w[:, nb * N_CORE:(nb + 1) * N_CORE]),
            "s": np.ascontiguousarray(s[:, nb * N_CORE:(nb + 1) * N_CORE]),
        })
    res = run_bass_kernel_spmd(
        _get_nc(), in_maps, core_ids=list(range(N_CORES)), trace=trace
    )
    out = np.empty((M_FULL, F), dtype=np.float32)
    for c in range(N_CORES):
        mb, nb = c % MSH, c // MSH
        out[mb * M_CORE:(mb + 1) * M_CORE, nb * N_CORE:(nb + 1) * N_CORE] = res.results[c]["o"]
    return out.reshape(B, S, F), res


def kernel(x, qkernel, qscale):
    try:
        out, _ = _run(x, qkernel, qscale, trace=False)
    except Exception:
        # One retry for transient device-side failures.
        out, _ = _run(x, qkernel, qscale, trace=False)
    return out


def kernel_traced(x, qkernel, qscale):
    out, res = _run(x, qkernel, qscale, trace=True)
    return out, res
